# revision 3
# baseline (speedup 1.0000x reference)
"""GNN message passing (gather + weighted segment-sum) on 8 Trainium2 cores.

out[n, :] = sum_{e : dst[e] == n} weight[e] * queue[src[e], :]

v3: single-term bf16 (gate is 2e-2; this lands ~3e-3).
  * per-(window, parity) dma_gather with trailing-pad trim (count register)
    rotated across 4 SWDGE queues — maximizes SDMA drain parallelism.
  * weighted one-hot built per WINDOW with two whole-window tensor_tensor
    passes (is_equal, mult) kept in the DVE 2-byte fast path by packing
    dstoff/weight as x4-repeated bf16 so every operand's last dim is
    packed (stride 1, len 4) instead of a step-0 broadcast.
  * TensorE accumulates H^T @ G_hi into a [128, 64] PSUM tile per window.
"""

import contextlib
import sys

sys.path.insert(0, "/opt/trn_rl_repo")

import ml_dtypes
import numpy as np

import concourse.bass as bass  # noqa: F401
import concourse.mybir as mybir
import concourse.tile as tile
from concourse import bacc
from concourse.bass_utils import run_bass_kernel_spmd

P = 128
N_CORES = 8

N_NODES = 50000
N_EDGES = 800000
D_FEAT = 64


def _plan(n_nodes, n_cores):
    n_windows = -(-n_nodes // P)
    wpc = -(-n_windows // n_cores)
    cw = max(d for d in range(1, min(wpc, 8) + 1) if wpc % d == 0)
    nchunk = wpc // cw
    return wpc, cw, nchunk


def _host_prep(weight, src, dst, n_nodes, wpc, cw, nchunk, n_cores):
    """Bucket edges by (core, window, src parity); pad uniformly.

    Returns (epw, nb, idx_hbm, aux_hbm, cnt_hbm):
      idx_hbm [n_cores, nchunk, 2, 128, cw*epw//16] int16 (dma_gather layout,
              trailing pads are -1 so the gather ucode trims them)
      aux_hbm [n_cores, nchunk, P, 2, 2*cw*nb, 4] bf16
              plane 0 = dstoff x4, plane 1 = weight x4;
              column (j*2 + h)*nb + k = window j, parity h, block k.
      cnt_hbm [n_cores, 1, nchunk*cw*2] int32 valid-edge counts
    """
    e = src.shape[0]
    src = np.asarray(src).astype(np.int64).reshape(-1)
    dst = np.asarray(dst).astype(np.int64).reshape(-1)
    wgt = np.asarray(weight, dtype=np.float32).reshape(-1)

    w = dst >> 7
    core = w // wpc
    lw = w - core * wpc
    half = src & 1
    hidx = (src >> 1).astype(np.int16)
    dstoff = (dst & 127).astype(np.float32)

    nbuckets = n_cores * wpc * 2
    key = (core * wpc + lw) * 2 + half
    order = np.lexsort((src, key))
    counts = np.bincount(key, minlength=nbuckets)
    epw = int(-(-max(int(counts.max()), 1) // P) * P)
    offs = np.zeros(nbuckets + 1, np.int64)
    np.cumsum(counts, out=offs[1:])
    skey = key[order]
    rank = np.arange(e, dtype=np.int64) - offs[skey]
    dest = skey * epw + rank

    bf = ml_dtypes.bfloat16
    idx_arr = np.zeros(nbuckets * epw, np.int16)
    dst_arr = np.zeros(nbuckets * epw, bf)
    w_arr = np.zeros(nbuckets * epw, bf)
    idx_arr[dest] = hidx[order]
    dst_arr[dest] = dstoff[order].astype(bf)
    w_arr[dest] = wgt[order].astype(bf)
    # odd (second) bucket of each window: mark trailing pads -1 for ucode trim
    i4 = idx_arr.reshape(nbuckets // 2, 2, epw)
    c2 = counts.reshape(nbuckets // 2, 2)
    for b in range(nbuckets // 2):
        i4[b, 1, c2[b, 1]:] = -1

    nb = epw // P
    big = cw * epw
    shp = (n_cores, nchunk, cw, 2, epw)
    idx_arr = idx_arr.reshape(shp)
    dst_arr = dst_arr.reshape(shp)
    w_arr = w_arr.reshape(shp)

    big2 = 2 * big
    a = idx_arr.reshape(n_cores, nchunk, big2 // 16, 16)
    a = a.transpose(0, 1, 3, 2)  # [.., 16, big2//16]
    idx_hbm = np.broadcast_to(
        a[:, :, None, :, :], (n_cores, nchunk, 8, 16, big2 // 16)
    ).reshape(n_cores, nchunk, P, big2 // 16)
    idx_hbm = np.ascontiguousarray(idx_hbm)

    def pack(x):
        # [core, chunk, P, col, 4] with col = (j*2+h)*nb + k, value x4-repeated
        y = x.reshape(n_cores, nchunk, cw, 2, nb, P)
        y = y.transpose(0, 1, 5, 2, 3, 4)  # [core, chunk, P, j, h, k]
        y = y.reshape(n_cores, nchunk, P, 2 * cw * nb)
        return np.broadcast_to(
            y[..., None], (n_cores, nchunk, P, 2 * cw * nb, 4)
        )

    aux_hbm = np.ascontiguousarray(
        np.stack([pack(dst_arr), pack(w_arr)], axis=3)
    )  # [core, chunk, P, 2, cols, 4]
    cnt_w = epw + counts.reshape(-1, 2)[:, 1]  # even section full + odd valid
    cnt_hbm = np.ascontiguousarray(
        cnt_w.reshape(n_cores, nchunk, cw)
        .reshape(n_cores, 1, nchunk * cw)
        .astype(np.int32)
    )
    return epw, nb, idx_hbm, aux_hbm, cnt_hbm


ALL_PARTS = frozenset({"gather", "dve", "mm", "out"})


def _build(n_nodes, d, epw, wpc, cw, nchunk, iters=1, parts=ALL_PARTS):
    f32 = mybir.dt.float32
    bf16 = mybir.dt.bfloat16
    nb = epw // P
    big = cw * epw
    bpc = cw * nb
    ncol = 2 * bpc
    ne = n_nodes // 2
    assert n_nodes % 2 == 0

    nc = bacc.Bacc(
        "TRN2", target_bir_lowering=False, debug=False, num_swdge_queues=4
    )

    qhl_t = nc.dram_tensor("qhl", [ne, 2 * d], bf16, kind="ExternalInput")
    big2 = 2 * big
    idx_t = nc.dram_tensor(
        "idx", [nchunk, P, big2 // 16], mybir.dt.int16, kind="ExternalInput"
    )
    aux_t = nc.dram_tensor(
        "aux", [nchunk, P, 2, ncol, 4], bf16, kind="ExternalInput"
    )
    iota_t = nc.dram_tensor("iota", [P, P], bf16, kind="ExternalInput")
    cnt_t = nc.dram_tensor(
        "cnt", [1, nchunk * cw], mybir.dt.int32, kind="ExternalInput"
    )
    out_t = nc.dram_tensor("out", [wpc * P, d], f32, kind="ExternalOutput")

    qpair = qhl_t.ap()[:, :]

    gbufs = 6
    with tile.TileContext(nc) as tc:
        with (
            tc.tile_pool(name="const", bufs=1) as cpool,
            tc.tile_pool(name="io", bufs=2) as iopool,
            tc.tile_pool(name="gat", bufs=gbufs) as gpool,
            tc.tile_pool(name="hot", bufs=3) as hpool,
            tc.tile_pool(name="ost", bufs=4) as opool,
            tc.tile_pool(name="ps", bufs=4, space="PSUM") as ppool,
        ):
            iota_f = cpool.tile([P, P], bf16)
            nc.sync.dma_start(out=iota_f[:], in_=iota_t.ap()[:, :])
            cnt = cpool.tile([1, nchunk * cw], mybir.dt.int32)
            nc.sync.dma_start(out=cnt[:], in_=cnt_t.ap()[:, :])
            # pre-zero gather slots: trimmed pad tails keep stale-but-finite
            # data (previously gathered queue rows) after the first iteration
            for _ in range(gbufs):
                gz = gpool.tile([P, 2 * nb, 2 * d], bf16, tag="g")
                nc.vector.memset(gz[:], 0)

            loop = tc.For_i(0, iters, 1) if iters > 1 else contextlib.nullcontext()
            with loop:
                for c in range(nchunk):
                    it = iopool.tile(
                        [P, big2 // 16], mybir.dt.int16, tag="idx"
                    )
                    nc.sync.dma_start(out=it[:], in_=idx_t.ap()[c])
                    aux = iopool.tile([P, 2, ncol, 4], bf16, tag="aux")
                    nc.sync.dma_start(out=aux[:], in_=aux_t.ap()[c])

                    for j in range(cw):
                        g = gpool.tile([P, 2 * nb, 2 * d], bf16, tag="g")
                        if "gather" in parts:
                            sl = 2 * epw // 16
                            gidx = c * cw + j
                            r = nc.alloc_register(mybir.EngineType.Pool)
                            nc.gpsimd.reg_load(r, cnt[0:1, gidx : gidx + 1])
                            nc.gpsimd.dma_gather(
                                out_ap=g[:],
                                in_ap=qpair,
                                idxs_ap=it[:, j * sl : (j + 1) * sl],
                                num_idxs=2 * epw,
                                num_idxs_reg=r,
                                elem_size=2 * d,
                                elem_step=2 * d,
                                single_packet=False,
                                queue_num=(c * cw + j) % 4,
                            )
                        elif "seqload" in parts:
                            flat = qhl_t.ap()[0 : P * 128, :].rearrange(
                                "(p c) d -> p (c d)", p=P
                            )
                            nc.sync.dma_start(
                                out=g[:].rearrange("p a b -> p (a b)"),
                                in_=flat[:, 0 : 2 * nb * 2 * d],
                            )

                        nbw = 2 * nb
                        wcol = j * nbw  # first aux column of this window

                        if "dve" in parts:
                            # H[p, b, j128] = w * (iota == dstoff), whole window
                            # in two passes; x4-repeat views keep 2x mode legal.
                            h01 = hpool.tile([P, nbw, P], bf16, tag="h01")
                            h01ap = h01[:]
                            o4 = bass.AP(
                                h01ap.tensor,
                                h01ap.offset,
                                [h01ap.ap[0], [P, nbw], [4, P // 4], [1, 4]],
                            )
                            iotap = iota_f[:]
                            iota4 = bass.AP(
                                iotap.tensor,
                                iotap.offset,
                                [iotap.ap[0], [0, nbw], [4, P // 4], [1, 4]],
                            )
                            dst_ap = aux[:, 0, wcol : wcol + nbw, :]
                            dst4 = bass.AP(
                                dst_ap.tensor,
                                dst_ap.offset,
                                [dst_ap.ap[0], [4, nbw], [0, P // 4], [1, 4]],
                            )
                            w_ap = aux[:, 1, wcol : wcol + nbw, :]
                            w4 = bass.AP(
                                w_ap.tensor,
                                w_ap.offset,
                                [w_ap.ap[0], [4, nbw], [0, P // 4], [1, 4]],
                            )
                            nc.vector.tensor_tensor(
                                out=o4,
                                in0=iota4,
                                in1=dst4,
                                op=mybir.AluOpType.is_equal,
                            )
                            nc.vector.tensor_tensor(
                                out=o4, in0=o4, in1=w4, op=mybir.AluOpType.mult
                            )
                        if "mm" in parts:
                            ps = ppool.tile([P, d], f32)
                            for bi in range(nbw):
                                h, k = divmod(bi, nb)
                                lhs = (
                                    h01[:, bi, :] if "dve" in parts else iota_f[:]
                                )
                                off = h * d
                                nc.tensor.matmul(
                                    ps[:],
                                    lhsT=lhs,
                                    rhs=g[:, bi, off : off + d],
                                    start=bi == 0,
                                    stop=bi == nbw - 1,
                                )
                        wg = c * cw + j
                        if "out" in parts and "mm" in parts:
                            ot = opool.tile([P, d], f32, tag="ot")
                            nc.scalar.copy(ot[:], ps[:])
                            nc.sync.dma_start(
                                out=out_t.ap()[wg * P : (wg + 1) * P, :], in_=ot[:]
                            )
                        elif "dve" in parts and "mm" not in parts:
                            nc.sync.dma_start(
                                out=out_t.ap()[wg * P : (wg + 1) * P, 0:d],
                                in_=h01[:, 0, 0 : 2 * d].bitcast(f32),
                            )
                        elif "dve" not in parts and "mm" not in parts:
                            nc.sync.dma_start(
                                out=out_t.ap()[wg * P : (wg + 1) * P, :],
                                in_=g[:, 0, :].bitcast(f32),
                            )
    nc.compile()
    return nc


def _make_inputs(queue, idx_hbm, aux_hbm, cnt_hbm, n_cores):
    bf = ml_dtypes.bfloat16
    q = np.asarray(queue, dtype=np.float32)
    hi = q.astype(bf)
    ne, d = q.shape[0] // 2, q.shape[1]
    qhl = np.empty((ne, 2 * d), bf)
    qhl[:, 0:d] = hi[0::2]
    qhl[:, d : 2 * d] = hi[1::2]
    iota_np = np.ascontiguousarray(
        np.broadcast_to(np.arange(P, dtype=np.float32), (P, P)).astype(bf)
    )
    return [
        {
            "qhl": qhl,
            "idx": idx_hbm[c],
            "aux": aux_hbm[c],
            "iota": iota_np,
            "cnt": cnt_hbm[c],
        }
        for c in range(n_cores)
    ]


def _run(queue, weight, src, dst, n_nodes, d, n_cores, trace=False, iters=1):
    queue = np.ascontiguousarray(np.asarray(queue, dtype=np.float32))
    wpc, cw, nchunk = _plan(n_nodes, n_cores)
    epw, nb, idx_hbm, aux_hbm, cnt_hbm = _host_prep(
        weight, src, dst, n_nodes, wpc, cw, nchunk, n_cores
    )
    nc = _build(n_nodes, d, epw, wpc, cw, nchunk, iters=iters)
    in_maps = _make_inputs(queue, idx_hbm, aux_hbm, cnt_hbm, n_cores)
    res = run_bass_kernel_spmd(nc, in_maps, core_ids=list(range(n_cores)), trace=trace)
    full = np.concatenate([res.results[c]["out"] for c in range(n_cores)], axis=0)
    return full[:n_nodes], res


def kernel(queue, weight, src, dst):
    out, _ = _run(queue, weight, src, dst, N_NODES, D_FEAT, N_CORES)
    return out


# revision 4
# speedup vs baseline: 1.1105x; 1.1105x over previous
"""GNN message passing (gather + weighted segment-sum) on 8 Trainium2 cores.

out[n, :] = sum_{e : dst[e] == n} weight[e] * queue[src[e], :]

v4: single-term bf16 (gate is 2e-2; this lands ~2.7e-3).
  * queue stored as bf16 pair-rows [hi(2p) | hi(2p+1)] (256 B each, the
    dma_gather minimum element) — 6.4 MB working set halves the random-read
    footprint vs a hi/lo split; an edge's parity picks the rhs column half.
  * ONE dma_gather per destination window covering both src-parity buckets
    (even bucket pads index row 0 with weight 0; odd tail pads are -1 and
    trimmed via the count register), rotated across 4 SWDGE queues.
  * weighted one-hot built per WINDOW with two whole-window tensor_tensor
    passes (is_equal, mult) kept in the DVE 2-byte fast path by packing
    dstoff/weight as x4-repeated bf16 so every operand's last dim is
    packed (stride 1, len 4) instead of a step-0 broadcast.
  * TensorE accumulates H^T @ G into a [128, 64] PSUM tile per window;
    block h*nb+k uses rhs columns [h*64, h*64+64) for the parity.
"""

import contextlib
import sys

sys.path.insert(0, "/opt/trn_rl_repo")

import ml_dtypes
import numpy as np

import concourse.bass as bass  # noqa: F401
import concourse.mybir as mybir
import concourse.tile as tile
from concourse import bacc
from concourse.bass_utils import run_bass_kernel_spmd

P = 128
N_CORES = 8

N_NODES = 50000
N_EDGES = 800000
D_FEAT = 64


def _plan(n_nodes, n_cores):
    n_windows = -(-n_nodes // P)
    wpc = -(-n_windows // n_cores)
    cw = max(d for d in range(1, min(wpc, 8) + 1) if wpc % d == 0)
    nchunk = wpc // cw
    return wpc, cw, nchunk


def _host_prep(weight, src, dst, n_nodes, wpc, cw, nchunk, n_cores):
    """Bucket edges by (core, window, src parity); pad uniformly.

    Returns (epw, nb, idx_hbm, aux_hbm, cnt_hbm):
      idx_hbm [n_cores, nchunk, 2, 128, cw*epw//16] int16 (dma_gather layout,
              trailing pads are -1 so the gather ucode trims them)
      aux_hbm [n_cores, nchunk, P, 2, 2*cw*nb, 4] bf16
              plane 0 = dstoff x4, plane 1 = weight x4;
              column (j*2 + h)*nb + k = window j, parity h, block k.
      cnt_hbm [n_cores, 1, nchunk*cw*2] int32 valid-edge counts
    """
    e = src.shape[0]
    src = np.asarray(src).astype(np.int64).reshape(-1)
    dst = np.asarray(dst).astype(np.int64).reshape(-1)
    wgt = np.asarray(weight, dtype=np.float32).reshape(-1)

    w = dst >> 7
    core = w // wpc
    lw = w - core * wpc
    half = src & 1
    hidx = (src >> 1).astype(np.int16)
    dstoff = (dst & 127).astype(np.float32)

    nbuckets = n_cores * wpc * 2
    key = (core * wpc + lw) * 2 + half
    order = np.lexsort((src, key))
    counts = np.bincount(key, minlength=nbuckets)
    epw = int(-(-max(int(counts.max()), 1) // P) * P)
    offs = np.zeros(nbuckets + 1, np.int64)
    np.cumsum(counts, out=offs[1:])
    skey = key[order]
    rank = np.arange(e, dtype=np.int64) - offs[skey]
    dest = skey * epw + rank

    bf = ml_dtypes.bfloat16
    idx_arr = np.zeros(nbuckets * epw, np.int16)
    dst_arr = np.zeros(nbuckets * epw, bf)
    w_arr = np.zeros(nbuckets * epw, bf)
    idx_arr[dest] = hidx[order]
    dst_arr[dest] = dstoff[order].astype(bf)
    w_arr[dest] = wgt[order].astype(bf)
    # odd (second) bucket of each window: mark trailing pads -1 for ucode trim
    i4 = idx_arr.reshape(nbuckets // 2, 2, epw)
    c2 = counts.reshape(nbuckets // 2, 2)
    for b in range(nbuckets // 2):
        i4[b, 1, c2[b, 1]:] = -1

    nb = epw // P
    big = cw * epw
    shp = (n_cores, nchunk, cw, 2, epw)
    idx_arr = idx_arr.reshape(shp)
    dst_arr = dst_arr.reshape(shp)
    w_arr = w_arr.reshape(shp)

    big2 = 2 * big
    a = idx_arr.reshape(n_cores, nchunk, big2 // 16, 16)
    a = a.transpose(0, 1, 3, 2)  # [.., 16, big2//16]
    idx_hbm = np.broadcast_to(
        a[:, :, None, :, :], (n_cores, nchunk, 8, 16, big2 // 16)
    ).reshape(n_cores, nchunk, P, big2 // 16)
    idx_hbm = np.ascontiguousarray(idx_hbm)

    def pack(x):
        # [core, chunk, P, col, 4] with col = (j*2+h)*nb + k, value x4-repeated
        y = x.reshape(n_cores, nchunk, cw, 2, nb, P)
        y = y.transpose(0, 1, 5, 2, 3, 4)  # [core, chunk, P, j, h, k]
        y = y.reshape(n_cores, nchunk, P, 2 * cw * nb)
        return np.broadcast_to(
            y[..., None], (n_cores, nchunk, P, 2 * cw * nb, 4)
        )

    aux_hbm = np.ascontiguousarray(
        np.stack([pack(dst_arr), pack(w_arr)], axis=3)
    )  # [core, chunk, P, 2, cols, 4]
    cnt_w = epw + counts.reshape(-1, 2)[:, 1]  # even section full + odd valid
    cnt_hbm = np.ascontiguousarray(
        cnt_w.reshape(n_cores, nchunk, cw)
        .reshape(n_cores, 1, nchunk * cw)
        .astype(np.int32)
    )
    return epw, nb, idx_hbm, aux_hbm, cnt_hbm


ALL_PARTS = frozenset({"gather", "dve", "mm", "out"})


def _build(n_nodes, d, epw, wpc, cw, nchunk, iters=1, parts=ALL_PARTS):
    f32 = mybir.dt.float32
    bf16 = mybir.dt.bfloat16
    nb = epw // P
    big = cw * epw
    bpc = cw * nb
    ncol = 2 * bpc
    ne = n_nodes // 2
    assert n_nodes % 2 == 0

    nc = bacc.Bacc(
        "TRN2", target_bir_lowering=False, debug=False, num_swdge_queues=4
    )

    qhl_t = nc.dram_tensor("qhl", [ne, 2 * d], bf16, kind="ExternalInput")
    big2 = 2 * big
    idx_t = nc.dram_tensor(
        "idx", [nchunk, P, big2 // 16], mybir.dt.int16, kind="ExternalInput"
    )
    aux_t = nc.dram_tensor(
        "aux", [nchunk, P, 2, ncol, 4], bf16, kind="ExternalInput"
    )
    iota_t = nc.dram_tensor("iota", [P, P], bf16, kind="ExternalInput")
    cnt_t = nc.dram_tensor(
        "cnt", [1, nchunk * cw], mybir.dt.int32, kind="ExternalInput"
    )
    out_t = nc.dram_tensor("out", [wpc * P, d], f32, kind="ExternalOutput")

    qpair = qhl_t.ap()[:, :]

    gbufs = 6
    with tile.TileContext(nc) as tc:
        with (
            tc.tile_pool(name="const", bufs=1) as cpool,
            tc.tile_pool(name="io", bufs=2) as iopool,
            tc.tile_pool(name="gat", bufs=gbufs) as gpool,
            tc.tile_pool(name="hot", bufs=3) as hpool,
            tc.tile_pool(name="ost", bufs=4) as opool,
            tc.tile_pool(name="ps", bufs=4, space="PSUM") as ppool,
        ):
            iota_f = cpool.tile([P, P], bf16)
            nc.sync.dma_start(out=iota_f[:], in_=iota_t.ap()[:, :])
            cnt = cpool.tile([1, nchunk * cw], mybir.dt.int32)
            nc.sync.dma_start(out=cnt[:], in_=cnt_t.ap()[:, :])
            # pre-zero gather slots: trimmed pad tails keep stale-but-finite
            # data (previously gathered queue rows) after the first iteration
            for _ in range(gbufs):
                gz = gpool.tile([P, 2 * nb, 2 * d], bf16, tag="g")
                nc.vector.memset(gz[:], 0)

            loop = tc.For_i(0, iters, 1) if iters > 1 else contextlib.nullcontext()
            with loop:
                for c in range(nchunk):
                    it = iopool.tile(
                        [P, big2 // 16], mybir.dt.int16, tag="idx"
                    )
                    nc.sync.dma_start(out=it[:], in_=idx_t.ap()[c])
                    aux = iopool.tile([P, 2, ncol, 4], bf16, tag="aux")
                    nc.sync.dma_start(out=aux[:], in_=aux_t.ap()[c])

                    for j in range(cw):
                        g = gpool.tile([P, 2 * nb, 2 * d], bf16, tag="g")
                        if "gather" in parts:
                            sl = 2 * epw // 16
                            gidx = c * cw + j
                            r = nc.alloc_register(mybir.EngineType.Pool)
                            nc.gpsimd.reg_load(r, cnt[0:1, gidx : gidx + 1])
                            nc.gpsimd.dma_gather(
                                out_ap=g[:],
                                in_ap=qpair,
                                idxs_ap=it[:, j * sl : (j + 1) * sl],
                                num_idxs=2 * epw,
                                num_idxs_reg=r,
                                elem_size=2 * d,
                                elem_step=2 * d,
                                single_packet=False,
                                queue_num=(c * cw + j) % 4,
                            )
                        elif "seqload" in parts:
                            flat = qhl_t.ap()[0 : P * 128, :].rearrange(
                                "(p c) d -> p (c d)", p=P
                            )
                            nc.sync.dma_start(
                                out=g[:].rearrange("p a b -> p (a b)"),
                                in_=flat[:, 0 : 2 * nb * 2 * d],
                            )

                        nbw = 2 * nb
                        wcol = j * nbw  # first aux column of this window

                        if "dve" in parts:
                            # H[p, b, j128] = w * (iota == dstoff), whole window
                            # in two passes; x4-repeat views keep 2x mode legal.
                            h01 = hpool.tile([P, nbw, P], bf16, tag="h01")
                            h01ap = h01[:]
                            o4 = bass.AP(
                                h01ap.tensor,
                                h01ap.offset,
                                [h01ap.ap[0], [P, nbw], [4, P // 4], [1, 4]],
                            )
                            iotap = iota_f[:]
                            iota4 = bass.AP(
                                iotap.tensor,
                                iotap.offset,
                                [iotap.ap[0], [0, nbw], [4, P // 4], [1, 4]],
                            )
                            dst_ap = aux[:, 0, wcol : wcol + nbw, :]
                            dst4 = bass.AP(
                                dst_ap.tensor,
                                dst_ap.offset,
                                [dst_ap.ap[0], [4, nbw], [0, P // 4], [1, 4]],
                            )
                            w_ap = aux[:, 1, wcol : wcol + nbw, :]
                            w4 = bass.AP(
                                w_ap.tensor,
                                w_ap.offset,
                                [w_ap.ap[0], [4, nbw], [0, P // 4], [1, 4]],
                            )
                            nc.vector.tensor_tensor(
                                out=o4,
                                in0=iota4,
                                in1=dst4,
                                op=mybir.AluOpType.is_equal,
                            )
                            nc.vector.tensor_tensor(
                                out=o4, in0=o4, in1=w4, op=mybir.AluOpType.mult
                            )
                        if "mm" in parts:
                            ps = ppool.tile([P, d], f32)
                            for bi in range(nbw):
                                h, k = divmod(bi, nb)
                                lhs = (
                                    h01[:, bi, :] if "dve" in parts else iota_f[:]
                                )
                                off = h * d
                                nc.tensor.matmul(
                                    ps[:],
                                    lhsT=lhs,
                                    rhs=g[:, bi, off : off + d],
                                    start=bi == 0,
                                    stop=bi == nbw - 1,
                                )
                        wg = c * cw + j
                        if "out" in parts and "mm" in parts:
                            ot = opool.tile([P, d], f32, tag="ot")
                            nc.scalar.copy(ot[:], ps[:])
                            nc.sync.dma_start(
                                out=out_t.ap()[wg * P : (wg + 1) * P, :], in_=ot[:]
                            )
                        elif "dve" in parts and "mm" not in parts:
                            nc.sync.dma_start(
                                out=out_t.ap()[wg * P : (wg + 1) * P, 0:d],
                                in_=h01[:, 0, 0 : 2 * d].bitcast(f32),
                            )
                        elif "dve" not in parts and "mm" not in parts:
                            nc.sync.dma_start(
                                out=out_t.ap()[wg * P : (wg + 1) * P, :],
                                in_=g[:, 0, :].bitcast(f32),
                            )
    nc.compile()
    return nc


def _make_inputs(queue, idx_hbm, aux_hbm, cnt_hbm, n_cores):
    bf = ml_dtypes.bfloat16
    q = np.asarray(queue, dtype=np.float32)
    hi = q.astype(bf)
    ne, d = q.shape[0] // 2, q.shape[1]
    qhl = np.empty((ne, 2 * d), bf)
    qhl[:, 0:d] = hi[0::2]
    qhl[:, d : 2 * d] = hi[1::2]
    iota_np = np.ascontiguousarray(
        np.broadcast_to(np.arange(P, dtype=np.float32), (P, P)).astype(bf)
    )
    return [
        {
            "qhl": qhl,
            "idx": idx_hbm[c],
            "aux": aux_hbm[c],
            "iota": iota_np,
            "cnt": cnt_hbm[c],
        }
        for c in range(n_cores)
    ]


def _run(queue, weight, src, dst, n_nodes, d, n_cores, trace=False, iters=1):
    queue = np.ascontiguousarray(np.asarray(queue, dtype=np.float32))
    wpc, cw, nchunk = _plan(n_nodes, n_cores)
    epw, nb, idx_hbm, aux_hbm, cnt_hbm = _host_prep(
        weight, src, dst, n_nodes, wpc, cw, nchunk, n_cores
    )
    nc = _build(n_nodes, d, epw, wpc, cw, nchunk, iters=iters)
    in_maps = _make_inputs(queue, idx_hbm, aux_hbm, cnt_hbm, n_cores)
    res = run_bass_kernel_spmd(nc, in_maps, core_ids=list(range(n_cores)), trace=trace)
    full = np.concatenate([res.results[c]["out"] for c in range(n_cores)], axis=0)
    return full[:n_nodes], res


def kernel(queue, weight, src, dst):
    out, _ = _run(queue, weight, src, dst, N_NODES, D_FEAT, N_CORES)
    return out


# revision 5
# speedup vs baseline: 1.1230x; 1.0113x over previous
"""GNN message passing (gather + weighted segment-sum) on 8 Trainium2 cores.

out[n, :] = sum_{e : dst[e] == n} weight[e] * queue[src[e], :]

v3: single-term bf16 (gate is 2e-2; this lands ~3e-3).
  * per-(window, parity) dma_gather with trailing-pad trim (count register)
    rotated across 4 SWDGE queues — maximizes SDMA drain parallelism.
  * weighted one-hot built per WINDOW with two whole-window tensor_tensor
    passes (is_equal, mult) kept in the DVE 2-byte fast path by packing
    dstoff/weight as x4-repeated bf16 so every operand's last dim is
    packed (stride 1, len 4) instead of a step-0 broadcast.
  * TensorE accumulates H^T @ G_hi into a [128, 64] PSUM tile per window.
"""

import contextlib
import sys

sys.path.insert(0, "/opt/trn_rl_repo")

import ml_dtypes
import numpy as np

import concourse.bass as bass  # noqa: F401
import concourse.mybir as mybir
import concourse.tile as tile
from concourse import bacc
from concourse.bass_utils import run_bass_kernel_spmd

P = 128
N_CORES = 8

N_NODES = 50000
N_EDGES = 800000
D_FEAT = 64


def _plan(n_nodes, n_cores):
    n_windows = -(-n_nodes // P)
    wpc = -(-n_windows // n_cores)
    cw = max(d for d in range(1, min(wpc, 8) + 1) if wpc % d == 0)
    nchunk = wpc // cw
    return wpc, cw, nchunk


def _host_prep(weight, src, dst, n_nodes, wpc, cw, nchunk, n_cores):
    """Bucket edges by (core, window, src parity); pad uniformly.

    Returns (epw, nb, idx_hbm, aux_hbm, cnt_hbm):
      idx_hbm [n_cores, nchunk, 2, 128, cw*epw//16] int16 (dma_gather layout,
              trailing pads are -1 so the gather ucode trims them)
      aux_hbm [n_cores, nchunk, P, 2, 2*cw*nb, 4] bf16
              plane 0 = dstoff x4, plane 1 = weight x4;
              column (j*2 + h)*nb + k = window j, parity h, block k.
      cnt_hbm [n_cores, 1, nchunk*cw*2] int32 valid-edge counts
    """
    e = src.shape[0]
    src = np.asarray(src).astype(np.int64).reshape(-1)
    dst = np.asarray(dst).astype(np.int64).reshape(-1)
    wgt = np.asarray(weight, dtype=np.float32).reshape(-1)

    w = dst >> 7
    core = w // wpc
    lw = w - core * wpc
    half = src & 1
    hidx = (src >> 1).astype(np.int16)
    dstoff = (dst & 127).astype(np.float32)

    nbuckets = n_cores * wpc * 2
    key = (core * wpc + lw) * 2 + half
    order = np.lexsort((src, key))
    counts = np.bincount(key, minlength=nbuckets)
    epw = int(-(-max(int(counts.max()), 1) // P) * P)
    offs = np.zeros(nbuckets + 1, np.int64)
    np.cumsum(counts, out=offs[1:])
    skey = key[order]
    rank = np.arange(e, dtype=np.int64) - offs[skey]
    dest = skey * epw + rank

    bf = ml_dtypes.bfloat16
    # pad slots point at pseudorandom spread rows (weight 0 kills their
    # contribution); a single shared pad row serializes on one HBM channel
    idx_arr = ((np.arange(nbuckets * epw, dtype=np.int64) * 9973) % (
        n_nodes // 2)).astype(np.int16)
    dst_arr = np.zeros(nbuckets * epw, bf)
    w_arr = np.zeros(nbuckets * epw, bf)
    idx_arr[dest] = hidx[order]
    dst_arr[dest] = dstoff[order].astype(bf)
    w_arr[dest] = wgt[order].astype(bf)
    # odd (second) bucket of each window: mark trailing pads -1 for ucode trim
    i4 = idx_arr.reshape(nbuckets // 2, 2, epw)
    c2 = counts.reshape(nbuckets // 2, 2)
    for b in range(nbuckets // 2):
        i4[b, 1, c2[b, 1]:] = -1

    nb = epw // P
    big = cw * epw
    shp = (n_cores, nchunk, cw, 2, epw)
    idx_arr = idx_arr.reshape(shp)
    dst_arr = dst_arr.reshape(shp)
    w_arr = w_arr.reshape(shp)

    big2 = 2 * big
    a = idx_arr.reshape(n_cores, nchunk, big2 // 16, 16)
    a = a.transpose(0, 1, 3, 2)  # [.., 16, big2//16]
    idx_hbm = np.broadcast_to(
        a[:, :, None, :, :], (n_cores, nchunk, 8, 16, big2 // 16)
    ).reshape(n_cores, nchunk, P, big2 // 16)
    idx_hbm = np.ascontiguousarray(idx_hbm)

    def pack(x):
        # [core, chunk, P, col, 4] with col = (j*2+h)*nb + k, value x4-repeated
        y = x.reshape(n_cores, nchunk, cw, 2, nb, P)
        y = y.transpose(0, 1, 5, 2, 3, 4)  # [core, chunk, P, j, h, k]
        y = y.reshape(n_cores, nchunk, P, 2 * cw * nb)
        return np.broadcast_to(
            y[..., None], (n_cores, nchunk, P, 2 * cw * nb, 4)
        )

    aux_hbm = np.ascontiguousarray(
        np.stack([pack(dst_arr), pack(w_arr)], axis=3)
    )  # [core, chunk, P, 2, cols, 4]
    cnt_w = epw + counts.reshape(-1, 2)[:, 1]  # even section full + odd valid
    cnt_hbm = np.ascontiguousarray(
        cnt_w.reshape(n_cores, nchunk, cw)
        .reshape(n_cores, 1, nchunk * cw)
        .astype(np.int32)
    )
    return epw, nb, idx_hbm, aux_hbm, cnt_hbm


ALL_PARTS = frozenset({"gather", "dve", "mm", "out"})


def _build(n_nodes, d, epw, wpc, cw, nchunk, iters=1, parts=ALL_PARTS):
    f32 = mybir.dt.float32
    bf16 = mybir.dt.bfloat16
    nb = epw // P
    big = cw * epw
    bpc = cw * nb
    ncol = 2 * bpc
    ne = n_nodes // 2
    assert n_nodes % 2 == 0

    nc = bacc.Bacc(
        "TRN2", target_bir_lowering=False, debug=False, num_swdge_queues=4
    )

    qhl_t = nc.dram_tensor("qhl", [ne, 2 * d], bf16, kind="ExternalInput")
    big2 = 2 * big
    idx_t = nc.dram_tensor(
        "idx", [nchunk, P, big2 // 16], mybir.dt.int16, kind="ExternalInput"
    )
    aux_t = nc.dram_tensor(
        "aux", [nchunk, P, 2, ncol, 4], bf16, kind="ExternalInput"
    )
    iota_t = nc.dram_tensor("iota", [P, P], bf16, kind="ExternalInput")
    cnt_t = nc.dram_tensor(
        "cnt", [1, nchunk * cw], mybir.dt.int32, kind="ExternalInput"
    )
    out_t = nc.dram_tensor("out", [wpc * P, d], f32, kind="ExternalOutput")

    qpair = qhl_t.ap()[:, :]

    gbufs = 8
    with tile.TileContext(nc) as tc:
        with (
            tc.tile_pool(name="const", bufs=1) as cpool,
            tc.tile_pool(name="io", bufs=2) as iopool,
            tc.tile_pool(name="gat", bufs=gbufs) as gpool,
            tc.tile_pool(name="hot", bufs=4) as hpool,
            tc.tile_pool(name="ost", bufs=4) as opool,
            tc.tile_pool(name="ps", bufs=6, space="PSUM") as ppool,
        ):
            iota_f = cpool.tile([P, P], bf16)
            nc.sync.dma_start(out=iota_f[:], in_=iota_t.ap()[:, :])
            cnt = cpool.tile([1, nchunk * cw], mybir.dt.int32)
            nc.sync.dma_start(out=cnt[:], in_=cnt_t.ap()[:, :])
            # pre-zero gather slots: trimmed pad tails keep stale-but-finite
            # data (previously gathered queue rows) after the first iteration
            for _ in range(gbufs):
                gz = gpool.tile([P, 2 * nb, 2 * d], bf16, tag="g")
                nc.vector.memset(gz[:], 0)

            loop = tc.For_i(0, iters, 1) if iters > 1 else contextlib.nullcontext()
            with loop:
                for c in range(nchunk):
                    it = iopool.tile(
                        [P, big2 // 16], mybir.dt.int16, tag="idx"
                    )
                    nc.sync.dma_start(out=it[:], in_=idx_t.ap()[c])
                    aux = iopool.tile([P, 2, ncol, 4], bf16, tag="aux")
                    nc.sync.dma_start(out=aux[:], in_=aux_t.ap()[c])

                    for j in range(cw):
                        g = gpool.tile([P, 2 * nb, 2 * d], bf16, tag="g")
                        if "gather" in parts:
                            sl = 2 * epw // 16
                            gidx = c * cw + j
                            r = nc.alloc_register(mybir.EngineType.Pool)
                            nc.gpsimd.reg_load(r, cnt[0:1, gidx : gidx + 1])
                            nc.gpsimd.dma_gather(
                                out_ap=g[:],
                                in_ap=qpair,
                                idxs_ap=it[:, j * sl : (j + 1) * sl],
                                num_idxs=2 * epw,
                                num_idxs_reg=r,
                                elem_size=2 * d,
                                elem_step=2 * d,
                                single_packet=False,
                                queue_num=(c * cw + j) % 4,
                            )
                        elif "seqload" in parts:
                            flat = qhl_t.ap()[0 : P * 128, :].rearrange(
                                "(p c) d -> p (c d)", p=P
                            )
                            nc.sync.dma_start(
                                out=g[:].rearrange("p a b -> p (a b)"),
                                in_=flat[:, 0 : 2 * nb * 2 * d],
                            )

                        nbw = 2 * nb
                        wcol = j * nbw  # first aux column of this window

                        if "dve" in parts:
                            # H[p, b, j128] = w * (iota == dstoff), whole window
                            # in two passes; x4-repeat views keep 2x mode legal.
                            h01 = hpool.tile([P, nbw, P], bf16, tag="h01")
                            h01ap = h01[:]
                            o4 = bass.AP(
                                h01ap.tensor,
                                h01ap.offset,
                                [h01ap.ap[0], [P, nbw], [4, P // 4], [1, 4]],
                            )
                            iotap = iota_f[:]
                            iota4 = bass.AP(
                                iotap.tensor,
                                iotap.offset,
                                [iotap.ap[0], [0, nbw], [4, P // 4], [1, 4]],
                            )
                            dst_ap = aux[:, 0, wcol : wcol + nbw, :]
                            dst4 = bass.AP(
                                dst_ap.tensor,
                                dst_ap.offset,
                                [dst_ap.ap[0], [4, nbw], [0, P // 4], [1, 4]],
                            )
                            w_ap = aux[:, 1, wcol : wcol + nbw, :]
                            w4 = bass.AP(
                                w_ap.tensor,
                                w_ap.offset,
                                [w_ap.ap[0], [4, nbw], [0, P // 4], [1, 4]],
                            )
                            nc.vector.tensor_tensor(
                                out=o4,
                                in0=iota4,
                                in1=dst4,
                                op=mybir.AluOpType.is_equal,
                            )
                            nc.vector.tensor_tensor(
                                out=o4, in0=o4, in1=w4, op=mybir.AluOpType.mult
                            )
                        if "mm" in parts:
                            ps = ppool.tile([P, d], f32)
                            for bi in range(nbw):
                                h, k = divmod(bi, nb)
                                lhs = (
                                    h01[:, bi, :] if "dve" in parts else iota_f[:]
                                )
                                off = h * d
                                nc.tensor.matmul(
                                    ps[:],
                                    lhsT=lhs,
                                    rhs=g[:, bi, off : off + d],
                                    start=bi == 0,
                                    stop=bi == nbw - 1,
                                )
                        wg = c * cw + j
                        if "out" in parts and "mm" in parts:
                            ot = opool.tile([P, d], f32, tag="ot")
                            nc.scalar.copy(ot[:], ps[:])
                            nc.sync.dma_start(
                                out=out_t.ap()[wg * P : (wg + 1) * P, :], in_=ot[:]
                            )
                        elif "dve" in parts and "mm" not in parts:
                            nc.sync.dma_start(
                                out=out_t.ap()[wg * P : (wg + 1) * P, 0:d],
                                in_=h01[:, 0, 0 : 2 * d].bitcast(f32),
                            )
                        elif "dve" not in parts and "mm" not in parts:
                            nc.sync.dma_start(
                                out=out_t.ap()[wg * P : (wg + 1) * P, :],
                                in_=g[:, 0, :].bitcast(f32),
                            )
    nc.compile()
    return nc


def _make_inputs(queue, idx_hbm, aux_hbm, cnt_hbm, n_cores):
    bf = ml_dtypes.bfloat16
    q = np.asarray(queue, dtype=np.float32)
    hi = q.astype(bf)
    ne, d = q.shape[0] // 2, q.shape[1]
    qhl = np.empty((ne, 2 * d), bf)
    qhl[:, 0:d] = hi[0::2]
    qhl[:, d : 2 * d] = hi[1::2]
    iota_np = np.ascontiguousarray(
        np.broadcast_to(np.arange(P, dtype=np.float32), (P, P)).astype(bf)
    )
    return [
        {
            "qhl": qhl,
            "idx": idx_hbm[c],
            "aux": aux_hbm[c],
            "iota": iota_np,
            "cnt": cnt_hbm[c],
        }
        for c in range(n_cores)
    ]


def _run(queue, weight, src, dst, n_nodes, d, n_cores, trace=False, iters=1):
    queue = np.ascontiguousarray(np.asarray(queue, dtype=np.float32))
    wpc, cw, nchunk = _plan(n_nodes, n_cores)
    epw, nb, idx_hbm, aux_hbm, cnt_hbm = _host_prep(
        weight, src, dst, n_nodes, wpc, cw, nchunk, n_cores
    )
    nc = _build(n_nodes, d, epw, wpc, cw, nchunk, iters=iters)
    in_maps = _make_inputs(queue, idx_hbm, aux_hbm, cnt_hbm, n_cores)
    res = run_bass_kernel_spmd(nc, in_maps, core_ids=list(range(n_cores)), trace=trace)
    full = np.concatenate([res.results[c]["out"] for c in range(n_cores)], axis=0)
    return full[:n_nodes], res


def kernel(queue, weight, src, dst):
    out, _ = _run(queue, weight, src, dst, N_NODES, D_FEAT, N_CORES)
    return out
